# revision 15
# baseline (speedup 1.0000x reference)
"""Causal self-attention (B=128, T=512, C=512, H=16) on 8 Trainium2 NeuronCores.

Sharding: data-parallel over batch — each core computes 16 of the 128
batch elements end-to-end; weights are replicated. No collectives.

Per-core kernel (Bass/Tile; matmul operands fp16, fp32 accumulation),
organized as a cross-batch software pipeline paced by the ACT engine
(exp is the irreducible bottleneck at ~23us/batch). Each batch runs in
4 "steps" (one per head-group g); at step (b, g) we emit:

  PE    : scores(b,g) K=32 4-way row-packed | QKV(b+1) chunk |
          yT(b,g-1) 2-way col-packed | proj(b-1) chunk (g>=2)
  ACT   : exp(b,g) on fused 2-head [128,2,n] PSUM tiles (scale and a
          2^-6 bias folded in to keep unnormalized yT inside fp16 range)
  GpSimd: causal masks via affine_select on 4-head [128,4,128] tiles,
          x fp32->fp16 casts, half the normalize multiplies
  DVE   : QKV/proj PSUM evacuations, yT evac to fp16, paired
          reciprocals, the other half of the normalize multiplies (4x
          fp16 mode)
  DMA   : x loads, XBAR SBUF->SBUF transposes of x (off the PE
          entirely), fused [128,T] reciprocal row broadcasts, output

yT layout trick: per head pair, v_aug = [v|ones] (lo=0) and [ones|v]
(lo=1) puts the two softmax denominators on ADJACENT psum rows 63/64
(one [2,T] reciprocal per pair) while the ones-columns ride along the
same E' stream (no extra PE streaming for denominators).
"""

import math
import sys

if "/opt/trn_rl_repo" not in sys.path:
    sys.path.insert(0, "/opt/trn_rl_repo")

import numpy as np

import concourse.tile as tile
from concourse import bacc, mybir
from concourse.bass_utils import run_bass_kernel_spmd
from concourse.masks import make_identity

F32 = mybir.dt.float32
F16 = mybir.dt.float16
AF = mybir.ActivationFunctionType
ALU = mybir.AluOpType

B, T, C, H, D = 128, 512, 512, 16, 32
P = 128
N_CORES = 8
BC = B // N_CORES       # 16 batches per core
NCBLK = C // P          # 4
NTBLK = T // P          # 4
NHG = H // 4            # 4 head groups of 4 heads
SCALE = 1.0 / math.sqrt(D)
EXP_BIAS = -6.0 * math.log(2.0)  # exp(s)*2^-6: keeps unnormalized yT in fp16
QR0 = [0, 128, 256, 384]  # q range start per k-block (exact causal)
NQKV = 12                 # 8 QK psum groups + 4 V psum groups


def _build_kernel(tc, out, x, w_attn, b_attn, w_proj, b_proj, b_count=BC):
    nc = tc.nc
    with (
        tc.tile_pool(name="const", bufs=1) as const_pool,
        tc.tile_pool(name="xf16", bufs=2) as xf16_pool,
        tc.tile_pool(name="xt", bufs=2) as xt_pool,
        tc.tile_pool(name="qt", bufs=2) as qt_pool,
        tc.tile_pool(name="kt", bufs=2) as kt_pool,
        tc.tile_pool(name="ep0", bufs=3) as ep0_pool,
        tc.tile_pool(name="ep1", bufs=3) as ep1_pool,
        tc.tile_pool(name="ep2", bufs=3) as ep2_pool,
        tc.tile_pool(name="ep3", bufs=3) as ep3_pool,
        tc.tile_pool(name="staged", bufs=5) as staged_pool,
        tc.tile_pool(name="rec", bufs=4) as rec_pool,
        tc.tile_pool(name="bcast", bufs=5) as bcast_pool,
        tc.tile_pool(name="ytn", bufs=3) as ytn_pool,
        tc.tile_pool(name="obuf", bufs=3) as o_pool,
        tc.tile_pool(name="s_psum", bufs=2, space="PSUM") as s_psum,
        tc.tile_pool(name="y_psum", bufs=2, space="PSUM") as y_psum,
        tc.tile_pool(name="mm_psum", bufs=2, space="PSUM") as mm_psum,
    ):
        ep_pools = [ep0_pool, ep1_pool, ep2_pool, ep3_pool]

        # ---------------- preamble: weights / biases / constants ----------
        onesf = const_pool.tile([P, P], F32, tag="onesf")
        nc.gpsimd.memset(onesf[:], 1.0)
        ones_r = const_pool.tile([P, P], F16, tag="ones_r")
        nc.vector.tensor_copy(ones_r[:], onesf[:])
        ebias = const_pool.tile([P, 1], F32, tag="ebias")
        nc.gpsimd.memset(ebias[:], EXP_BIAS)
        identity16 = const_pool.tile([P, P], F16, tag="identity16")
        make_identity(nc, identity16[:])

        wqk = const_pool.tile([P, NCBLK, 2 * C], F16, tag="wqk")
        wv = const_pool.tile([P, NCBLK, C], F16, tag="wv")
        wp = const_pool.tile([P, NCBLK, C], F16, tag="wp")
        battn = const_pool.tile([P, 8], F32, tag="battn")
        bv_row = const_pool.tile([1, C], F16, tag="bv_row")
        bp_row = const_pool.tile([1, C], F16, tag="bp_row")
        with tc.tile_pool(name="stage", bufs=2) as stage_pool:
            for co in range(NCBLK):
                st = stage_pool.tile([P, 2 * C], F32, tag="stage", name="st_qk")
                nc.sync.dma_start(st[:], w_attn[co * P : (co + 1) * P, : 2 * C])
                nc.vector.tensor_copy(wqk[:, co, :], st[:])
                st2 = stage_pool.tile([P, 2 * C], F32, tag="stage", name="st_v")
                nc.sync.dma_start(st2[:, :C], w_attn[co * P : (co + 1) * P, 2 * C :])
                nc.vector.tensor_copy(wv[:, co, :], st2[:, :C])
                st3 = stage_pool.tile([P, 2 * C], F32, tag="stage", name="st_p")
                nc.sync.dma_start(st3[:, :C], w_proj[co * P : (co + 1) * P, :])
                nc.vector.tensor_copy(wp[:, co, :], st3[:, :C])

            nc.sync.dma_start(
                battn[:], b_attn[: 2 * C].rearrange("(g p) -> p g", p=P)
            )
            st4 = stage_pool.tile([P, 2 * C], F32, tag="stage", name="st_b")
            nc.sync.dma_start(
                st4[0:1, :C], b_attn[2 * C :].rearrange("(o f) -> o f", o=1)
            )
            nc.sync.dma_start(
                st4[0:1, C : 2 * C], b_proj.rearrange("(o f) -> o f", o=1)
            )
            nc.vector.tensor_copy(bv_row[:], st4[0:1, :C])
            nc.vector.tensor_copy(bp_row[:], st4[0:1, C : 2 * C])

        bv_bcast = const_pool.tile([P, C], F32, tag="bv_bcast")
        bp_bcast = const_pool.tile([P, C], F32, tag="bp_bcast")
        for row, bcast in ((bv_row, bv_bcast), (bp_row, bp_bcast)):
            ps = mm_psum.tile([P, C], F32, tag="mm", name="bps")
            nc.tensor.matmul(ps[:], ones_r[0:1, :], row[:], start=True, stop=True)
            nc.scalar.copy(bcast[:], ps[:])

        # v_aug double buffer: [128, tb, (g, idx), 64].  idx 0,1 ("v-first"):
        # [v | ones]; idx 2,3 ("ones-first"): [ones | v].  Pairs (idx0,idx2)
        # and (idx1,idx3) of each group share a yT psum tile so their
        # denominators land on adjacent rows 63/64.
        v_bufs = []
        for vb in range(2):
            v_buf = const_pool.tile([P, NTBLK, NHG, 4, 2 * D], F16, tag=f"vbuf{vb}")
            nc.gpsimd.memset(v_buf[:, :, :, 0:2, D : 2 * D], 1.0)
            nc.gpsimd.memset(v_buf[:, :, :, 2:4, 0:D], 1.0)
            v_bufs.append(v_buf)

        # ---------------- per-stage state ----------------
        xf16 = {}     # b -> [128, 4, 512] f16 (natural layout, host-cast)
        xt = {}       # b -> [128, 4, 512] f16 (c on partitions)
        qt = {}       # b -> [128, NHG, T]
        kt = {}
        ep = {}       # (b, g, i) -> ep tile [128, 4, n_i]
        yps = {}      # (b, g, pair) -> psum tile
        staged = {}   # (b, g, pair) -> staged fp16 [128, T]
        bcast = {}    # (b, g, pair) -> [128, T] f32 (1/den rows 0-31, 96-127)
        ytn = {}      # b -> [128, NCBLK, T] f16

        def emit_x_load(b):
            t = xf16_pool.tile([P, NTBLK, C], F16, tag="xf16", name="xf16t")
            nc.sync.dma_start(
                t[:], x[b].rearrange("(tb p) c -> p tb c", p=P)
            )
            xf16[b] = t

        def emit_x_transp(b, tbs):
            # PE fp16 transpose blocks (4 per tb into one psum bank), one
            # strided DVE evac per tb into the xt layout.
            if b not in xt:
                xt[b] = xt_pool.tile([P, NCBLK, T], F16, tag="xt", name="xtt")
            for tb in tbs:
                tps = mm_psum.tile([P, NCBLK, P], F16, tag="mm", name="tps")
                for cb in range(NCBLK):
                    nc.tensor.transpose(
                        tps[:, cb, :],
                        xf16[b][:, tb, cb * P : (cb + 1) * P],
                        identity16[:],
                    )
                nc.vector.tensor_copy(
                    xt[b][:, :, tb * P : (tb + 1) * P], tps[:]
                )

        def emit_qkv_group(b, k):
            # k in 0..7: QK psum group; 8..11: V group (tb = k-8)
            if b not in qt:
                qt[b] = qt_pool.tile([P, NHG, T], F16, tag="qt", name="qtt")
                kt[b] = kt_pool.tile([P, NHG, T], F16, tag="kt", name="ktt")
            if k < 8:
                ps = mm_psum.tile([P, T], F32, tag="mm", name="qkps")
                for co in range(NCBLK):
                    nc.tensor.matmul(
                        ps[:],
                        wqk[:, co, k * P : (k + 1) * P],
                        xt[b][:, co, :],
                        start=(co == 0),
                        stop=(co == NCBLK - 1),
                    )
                dst = qt[b][:, k, :] if k < NHG else kt[b][:, k - NHG, :]
                nc.vector.tensor_scalar_add(dst, ps[:], battn[:, k : k + 1])
            else:
                tb = k - 8
                ps = mm_psum.tile([P, C], F32, tag="mm", name="vps")
                for co in range(NCBLK):
                    nc.tensor.matmul(
                        ps[:],
                        xt[b][:, co, tb * P : (tb + 1) * P],
                        wv[:, co, :],
                        start=(co == 0),
                        stop=(co == NCBLK - 1),
                    )
                v_buf = v_bufs[b % 2]
                psr = ps.rearrange("p (g i d) -> p g i d", g=NHG, i=4)
                bvr = bv_bcast.rearrange("p (g i d) -> p g i d", g=NHG, i=4)
                nc.vector.tensor_tensor(
                    v_buf[:, tb, :, 0:2, 0:D], psr[:, :, 0:2, :],
                    bvr[:, :, 0:2, :], ALU.add,
                )
                nc.vector.tensor_tensor(
                    v_buf[:, tb, :, 2:4, D : 2 * D], psr[:, :, 2:4, :],
                    bvr[:, :, 2:4, :], ALU.add,
                )

        def emit_scores_i(b, g, i):
            # 4-way row-packed K=32 scores matmuls for block i, then exp
            # (2 heads per op) and the diagonal causal mask (all 4 heads).
            q0 = QR0[i]
            n = T - q0
            s01 = s_psum.tile([P, 2, T], F32, tag="s", name="s01")
            s23 = s_psum.tile([P, 2, T], F32, tag="s", name="s23")
            for idx in range(4):
                st = s01 if idx < 2 else s23
                nc.tensor.matmul(
                    st[:, idx % 2, :n],
                    kt[b][32 * idx : 32 * idx + 32, g, i * P : (i + 1) * P],
                    qt[b][32 * idx : 32 * idx + 32, g, q0:T],
                    start=True,
                    stop=True,
                    tile_position=(32 * idx, 0),
                )
            ept = ep_pools[i].tile([P, 4, n], F16, tag=f"ep{i}", name="ept")
            nc.scalar.activation(
                ept[:, 0:2, :], s01[:, :, :n], AF.Exp, scale=SCALE, bias=ebias[:]
            )
            nc.scalar.activation(
                ept[:, 2:4, :], s23[:, :, :n], AF.Exp, scale=SCALE, bias=ebias[:]
            )
            # causal mask on the diagonal block (local cols [0, 128)):
            # keep iff col - p >= 0, same for all 4 heads (GpSimd; DVE is
            # the saturated engine).
            nc.gpsimd.affine_select(
                out=ept[:, :, 0:P],
                in_=ept[:, :, 0:P],
                compare_op=ALU.is_ge,
                fill=0.0,
                base=0,
                channel_multiplier=-1,
                pattern=[[0, 4], [1, P]],
            )
            ep[(b, g, i)] = ept

        def emit_yt_pair(b, g, pair):
            # pair 0: heads (g,0)&(g,2); pair 1: heads (g,1)&(g,3).
            # lo=0 stationary [v|ones] -> rows 0-31 yT_A, 32-63 den_A;
            # lo=1 stationary [ones|v] -> rows 64-95 den_B, 96-127 yT_B.
            v_buf = v_bufs[b % 2]
            t = y_psum.tile([P, T], F32, tag="y", name="yps")
            for i in range(NTBLK):
                for lo, idx in ((0, pair), (1, pair + 2)):
                    nc.tensor.matmul(
                        t[64 * lo : 64 * lo + 64, QR0[i] : T],
                        v_buf[:, i, g, idx, :],
                        ep[(b, g, i)][:, idx, :],
                        start=(i == 0),
                        stop=(i == NTBLK - 1),
                        tile_position=(0, 64 * lo),
                        skip_group_check=True,
                    )
            yps[(b, g, pair)] = t

        def emit_evac_recip(b, g):
            # evac both pairs' yps to fp16 + paired reciprocal of the
            # adjacent denominator rows 63/64 (the custom DVE op must
            # keep in/out on the same partitions, hence the [66, T] rec
            # tile), then one broadcast DMA per pair replicates each rec
            # row to 32 partitions of the [128, T] bcast tile.
            for pair in (0, 1):
                t = yps.pop((b, g, pair))
                st = staged_pool.tile([P, T], F16, tag="staged", name="stt")
                nc.vector.tensor_copy(st[:], t[:])
                # full-tile reciprocal: the custom DVE op needs identical
                # in/out partition ranges; cost is free-size-based so the
                # extra (unused) rows are free.  Only rows 63/64 (the two
                # denominators) are consumed.
                r = rec_pool.tile([P, T], F32, tag="rec", name="rt")
                nc.vector.reciprocal_approx_fast(r[:, :], t[:, :])
                staged[(b, g, pair)] = st
                # bcast rows aligned with the staged yT rows (DVE needs
                # equal base partitions when both inputs are SBUF):
                # rows 0-31 <- 1/den_A, rows 96-127 <- 1/den_B.
                bc = bcast_pool.tile([P, T], F32, tag="bcast", name="bct")
                for lo, row in ((0, 63), (96, 64)):
                    src = (
                        r[row : row + 1, :]
                        .rearrange("r (a t) -> r a t", a=1)
                        .to_broadcast([1, 32, T])
                    )
                    nc.sync.dma_start(bc[lo : lo + 32, :], src)
                bcast[(b, g, pair)] = bc

        def emit_mults(b, g):
            # normalize yT by 1/den and write straight into ytn layout.
            # rec16 rows: 0 -> head (g,0), 1 -> (g,2), 2 -> (g,1), 3 -> (g,3)
            if b not in ytn:
                ytn[b] = ytn_pool.tile([P, NCBLK, T], F16, tag="ytn", name="ytnt")
            yt = ytn[b]
            bc0 = bcast[(b, g, 0)]
            bc1 = bcast[(b, g, 1)]
            st0 = staged[(b, g, 0)]
            st1 = staged[(b, g, 1)]
            # normalize multiplies mostly on GpSimd (SBUF-only operands);
            # 3 per batch go to DVE to even out the engine loads.
            eng0 = nc.vector if g < 3 else nc.gpsimd
            eng0.tensor_tensor(
                yt[0:32, g, :], st0[0:32, :], bc0[0:32, :], ALU.mult
            )
            nc.gpsimd.tensor_tensor(
                yt[64:96, g, :], st0[96:128, :], bc0[96:128, :], ALU.mult
            )
            nc.gpsimd.tensor_tensor(
                yt[32:64, g, :], st1[0:32, :], bc1[0:32, :], ALU.mult
            )
            nc.gpsimd.tensor_tensor(
                yt[96:128, g, :], st1[96:128, :], bc1[96:128, :], ALU.mult
            )
            del staged[(b, g, 0)], staged[(b, g, 1)]
            del bcast[(b, g, 0)], bcast[(b, g, 1)]

        def emit_proj(b, tb):
            ps = mm_psum.tile([P, C], F32, tag="mm", name="ops")
            for cb in range(NCBLK):
                nc.tensor.matmul(
                    ps[:],
                    ytn[b][:, cb, tb * P : (tb + 1) * P],
                    wp[:, cb, :],
                    start=(cb == 0),
                    stop=(cb == NCBLK - 1),
                )
            ob = o_pool.tile([P, C], F32, tag="obuf")
            nc.vector.tensor_tensor(ob[:], ps[:], bp_bcast[:], ALU.add)
            nc.sync.dma_start(out[b, tb * P : (tb + 1) * P, :], ob[:])

        # ---------------- prologue ----------------
        emit_x_load(0)
        emit_x_transp(0, [0, 1, 2, 3])
        emit_x_load(1)
        emit_x_transp(1, [0, 1, 2, 3])
        for k in range(NQKV):
            emit_qkv_group(0, k)

        # ---------------- main pipeline ----------------
        # stage s = 4*b + g; at step s:
        #   scores/exp/mask (s) | qkv(b+1) | yT (s-1) | evac+recip (s-1)
        #   mults (s-2) | proj(b-1) at g in {2,3} | x machinery for b+2
        n_steps = 4 * b_count

        def run_step(s):
            b, g = divmod(s, 4)
            pb, pg = divmod(s - 1, 4)   # previous stage (yT / evac)
            mb, mg = divmod(s - 2, 4)   # mults stage

            if s >= 2:
                emit_mults(mb, mg)

            qkv_sched = {0: [0, 1, 2], 1: [3, 4, 5], 2: [6, 7, 8], 3: [9, 10, 11]}
            chunks = qkv_sched[g] if b + 1 < b_count else []

            # PE order: each scores pack is separated by >=1.5us of
            # independent PE work so the exp WAR (2 s-tile psum rotation)
            # never blocks the in-order PE queue.
            emit_scores_i(b, g, 0)
            if chunks:
                emit_qkv_group(b + 1, chunks[0])
                emit_qkv_group(b + 1, chunks[1])
            emit_scores_i(b, g, 1)
            if chunks:
                emit_qkv_group(b + 1, chunks[2])
            if b >= 2:
                emit_proj(b - 2, g)
            emit_scores_i(b, g, 2)
            if s >= 1:
                emit_yt_pair(pb, pg, 0)
            emit_scores_i(b, g, 3)
            if s >= 1:
                emit_yt_pair(pb, pg, 1)
                emit_evac_recip(pb, pg)
                if pg == 3:
                    for i in range(NTBLK):
                        del ep[(pb, 3, i)]
            if g == 0 and b + 2 < b_count:
                emit_x_load(b + 2)
            if g == 2 and b + 2 < b_count:
                emit_x_transp(b + 2, [0, 1])
            if g == 3 and b + 2 < b_count:
                emit_x_transp(b + 2, [2, 3])
            if pg == 3 and pb >= 0:
                del qt[pb], kt[pb]

        for s in range(n_steps):
            run_step(s)

        # ---------------- epilogue ----------------
        lb = b_count - 1
        emit_yt_pair(lb, 3, 0)
        emit_yt_pair(lb, 3, 1)
        emit_evac_recip(lb, 3)
        emit_mults(lb, 2)
        emit_mults(lb, 3)
        for tb in range(NTBLK):
            emit_proj(lb - 1, tb)
        for tb in range(NTBLK):
            emit_proj(lb, tb)


_NC_CACHE = None


def build_nc(b_count: int = BC, num_devices: int = N_CORES):
    nc = bacc.Bacc(
        "TRN2", target_bir_lowering=False, debug=False, num_devices=num_devices
    )
    x = nc.dram_tensor("x", [b_count, T, C], F16, kind="ExternalInput").ap()
    w_attn = nc.dram_tensor("w_attn", [C, 3 * C], F32, kind="ExternalInput").ap()
    b_attn = nc.dram_tensor("b_attn", [3 * C], F32, kind="ExternalInput").ap()
    w_proj = nc.dram_tensor("w_proj", [C, C], F32, kind="ExternalInput").ap()
    b_proj = nc.dram_tensor("b_proj", [C], F32, kind="ExternalInput").ap()
    out = nc.dram_tensor("out", [b_count, T, C], F32, kind="ExternalOutput").ap()
    with tile.TileContext(nc) as tc:
        _build_kernel(tc, out, x, w_attn, b_attn, w_proj, b_proj, b_count)
    nc.compile()
    return nc


def _get_nc():
    global _NC_CACHE
    if _NC_CACHE is None:
        _NC_CACHE = build_nc(BC, N_CORES)
    return _NC_CACHE


def kernel(x, W_attn, b_attn, W_proj, b_proj):
    # host-side cast: the device consumes x in fp16 (matmul operand
    # precision) so there is no on-chip cast or fp32 x traffic at all.
    x = np.ascontiguousarray(np.asarray(x, dtype=np.float16))
    W_attn = np.ascontiguousarray(np.asarray(W_attn, dtype=np.float32))
    b_attn = np.ascontiguousarray(np.asarray(b_attn, dtype=np.float32))
    W_proj = np.ascontiguousarray(np.asarray(W_proj, dtype=np.float32))
    b_proj = np.ascontiguousarray(np.asarray(b_proj, dtype=np.float32))

    nc = _get_nc()
    in_maps = [
        {
            "x": x[c * BC : (c + 1) * BC],
            "w_attn": W_attn,
            "b_attn": b_attn,
            "w_proj": W_proj,
            "b_proj": b_proj,
        }
        for c in range(N_CORES)
    ]
    res = run_bass_kernel_spmd(nc, in_maps, core_ids=list(range(N_CORES)))
    return np.concatenate([res.results[c]["out"] for c in range(N_CORES)], axis=0)


# revision 19
# speedup vs baseline: 1.0849x; 1.0849x over previous
"""Causal self-attention (B=128, T=512, C=512, H=16) on 8 Trainium2 NeuronCores.

Sharding: data-parallel over batch — each core computes 16 of the 128
batch elements end-to-end; weights are replicated. No collectives.

Per-core kernel (Bass/Tile; matmul operands fp16, fp32 accumulation),
organized as a cross-batch software pipeline paced by the ACT engine
(exp is the irreducible bottleneck at ~23us/batch). Each batch runs in
4 "steps" (one per head-group g); at step (b, g) we emit:

  PE    : scores(b,g) K=32 4-way row-packed | QKV(b+1) chunk |
          yT(b,g-1) 2-way col-packed | proj(b-1) chunk (g>=2)
  ACT   : exp(b,g) on fused 2-head [128,2,n] PSUM tiles (scale and a
          2^-6 bias folded in to keep unnormalized yT inside fp16 range)
  GpSimd: causal masks via affine_select on 4-head [128,4,128] tiles,
          x fp32->fp16 casts, half the normalize multiplies
  DVE   : QKV/proj PSUM evacuations, yT evac to fp16, paired
          reciprocals, the other half of the normalize multiplies (4x
          fp16 mode)
  DMA   : x loads, XBAR SBUF->SBUF transposes of x (off the PE
          entirely), fused [128,T] reciprocal row broadcasts, output

yT layout trick: per head pair, v_aug = [v|ones] (lo=0) and [ones|v]
(lo=1) puts the two softmax denominators on ADJACENT psum rows 63/64
(one [2,T] reciprocal per pair) while the ones-columns ride along the
same E' stream (no extra PE streaming for denominators).
"""

import math
import sys

if "/opt/trn_rl_repo" not in sys.path:
    sys.path.insert(0, "/opt/trn_rl_repo")

import numpy as np

import concourse.tile as tile
from concourse import bacc, mybir
from concourse.bass_utils import run_bass_kernel_spmd
from concourse.masks import make_identity

F32 = mybir.dt.float32
F16 = mybir.dt.float16
AF = mybir.ActivationFunctionType
ALU = mybir.AluOpType

B, T, C, H, D = 128, 512, 512, 16, 32
P = 128
N_CORES = 8
BC = B // N_CORES       # 16 batches per core
NCBLK = C // P          # 4
NTBLK = T // P          # 4
NHG = H // 4            # 4 head groups of 4 heads
SCALE = 1.0 / math.sqrt(D)
EXP_BIAS = -6.0 * math.log(2.0)  # exp(s)*2^-6: keeps unnormalized yT in fp16
QR0 = [0, 128, 256, 384]  # q range start per k-block (exact causal)
NQKV = 12                 # 8 QK psum groups + 4 V psum groups


def _build_kernel(tc, out, x, w_attn, b_attn, w_proj, b_proj, b_count=BC):
    nc = tc.nc
    with (
        tc.tile_pool(name="const", bufs=1) as const_pool,
        tc.tile_pool(name="xf16", bufs=2) as xf16_pool,
        tc.tile_pool(name="xt", bufs=2) as xt_pool,
        tc.tile_pool(name="qt", bufs=2) as qt_pool,
        tc.tile_pool(name="kt", bufs=2) as kt_pool,
        tc.tile_pool(name="ep0", bufs=3) as ep0_pool,
        tc.tile_pool(name="ep1", bufs=3) as ep1_pool,
        tc.tile_pool(name="ep2", bufs=3) as ep2_pool,
        tc.tile_pool(name="ep3", bufs=3) as ep3_pool,
        tc.tile_pool(name="staged", bufs=5) as staged_pool,
        tc.tile_pool(name="rec", bufs=4) as rec_pool,
        tc.tile_pool(name="bcast", bufs=5) as bcast_pool,
        tc.tile_pool(name="ytn", bufs=3) as ytn_pool,
        tc.tile_pool(name="obuf", bufs=3) as o_pool,
        tc.tile_pool(name="s_psum", bufs=2, space="PSUM") as s_psum,
        tc.tile_pool(name="y_psum", bufs=2, space="PSUM") as y_psum,
        tc.tile_pool(name="mm_psum", bufs=2, space="PSUM") as mm_psum,
    ):
        ep_pools = [ep0_pool, ep1_pool, ep2_pool, ep3_pool]

        # ---------------- preamble: weights / biases / constants ----------
        onesf = const_pool.tile([P, P], F32, tag="onesf")
        nc.gpsimd.memset(onesf[:], 1.0)
        ones_r = const_pool.tile([P, P], F16, tag="ones_r")
        nc.vector.tensor_copy(ones_r[:], onesf[:])
        ebias = const_pool.tile([P, 1], F32, tag="ebias")
        nc.gpsimd.memset(ebias[:], EXP_BIAS)
        identity16 = const_pool.tile([P, P], F16, tag="identity16")
        make_identity(nc, identity16[:])
        # causal keep-mask for the diagonal block: tri[p, :, j] = (j >= p)
        tri4 = const_pool.tile([P, 4, P], F16, tag="tri4")
        nc.gpsimd.memset(tri4[:], 1.0)
        nc.gpsimd.affine_select(
            out=tri4[:],
            in_=tri4[:],
            compare_op=ALU.is_ge,
            fill=0.0,
            base=0,
            channel_multiplier=-1,
            pattern=[[0, 4], [1, P]],
        )

        wqk = const_pool.tile([P, NCBLK, 2 * C], F16, tag="wqk")
        wv = const_pool.tile([P, NCBLK, C], F16, tag="wv")
        wp = const_pool.tile([P, NCBLK, C], F16, tag="wp")
        battn = const_pool.tile([P, 8], F32, tag="battn")
        bv_row = const_pool.tile([1, C], F16, tag="bv_row")
        bp_row = const_pool.tile([1, C], F16, tag="bp_row")
        with tc.tile_pool(name="stage", bufs=2) as stage_pool:
            for co in range(NCBLK):
                st = stage_pool.tile([P, 2 * C], F32, tag="stage", name="st_qk")
                nc.sync.dma_start(st[:], w_attn[co * P : (co + 1) * P, : 2 * C])
                nc.vector.tensor_copy(wqk[:, co, :], st[:])
                st2 = stage_pool.tile([P, 2 * C], F32, tag="stage", name="st_v")
                nc.sync.dma_start(st2[:, :C], w_attn[co * P : (co + 1) * P, 2 * C :])
                nc.vector.tensor_copy(wv[:, co, :], st2[:, :C])
                st3 = stage_pool.tile([P, 2 * C], F32, tag="stage", name="st_p")
                nc.sync.dma_start(st3[:, :C], w_proj[co * P : (co + 1) * P, :])
                nc.vector.tensor_copy(wp[:, co, :], st3[:, :C])

            nc.sync.dma_start(
                battn[:], b_attn[: 2 * C].rearrange("(g p) -> p g", p=P)
            )
            st4 = stage_pool.tile([P, 2 * C], F32, tag="stage", name="st_b")
            nc.sync.dma_start(
                st4[0:1, :C], b_attn[2 * C :].rearrange("(o f) -> o f", o=1)
            )
            nc.sync.dma_start(
                st4[0:1, C : 2 * C], b_proj.rearrange("(o f) -> o f", o=1)
            )
            nc.vector.tensor_copy(bv_row[:], st4[0:1, :C])
            nc.vector.tensor_copy(bp_row[:], st4[0:1, C : 2 * C])

        bv_bcast = const_pool.tile([P, C], F16, tag="bv_bcast")
        bp_bcast = const_pool.tile([P, C], F16, tag="bp_bcast")
        for row, bcast in ((bv_row, bv_bcast), (bp_row, bp_bcast)):
            ps = mm_psum.tile([P, C], F32, tag="mm", name="bps")
            nc.tensor.matmul(ps[:], ones_r[0:1, :], row[:], start=True, stop=True)
            nc.scalar.copy(bcast[:], ps[:])

        # v_aug double buffer: [128, tb, (g, idx), 64].  idx 0,1 ("v-first"):
        # [v | ones]; idx 2,3 ("ones-first"): [ones | v].  Pairs (idx0,idx2)
        # and (idx1,idx3) of each group share a yT psum tile so their
        # denominators land on adjacent rows 63/64.
        v_bufs = []
        for vb in range(2):
            v_buf = const_pool.tile([P, NTBLK, NHG, 4, 2 * D], F16, tag=f"vbuf{vb}")
            nc.gpsimd.memset(v_buf[:, :, :, 0:2, D : 2 * D], 1.0)
            nc.gpsimd.memset(v_buf[:, :, :, 2:4, 0:D], 1.0)
            v_bufs.append(v_buf)

        # ---------------- per-stage state ----------------
        xf16 = {}     # b -> [128, 4, 512] f16 (natural layout, host-cast)
        xt = {}       # b -> [128, 4, 512] f16 (c on partitions)
        qt = {}       # b -> [128, NHG, T]
        kt = {}
        ep = {}       # (b, g, i) -> ep tile [128, 4, n_i]
        yps = {}      # (b, g, pair) -> psum tile
        staged = {}   # (b, g, pair) -> staged fp16 [128, T]
        bcast = {}    # (b, g, pair) -> [128, T] f32 (1/den rows 0-31, 96-127)
        ytn = {}      # b -> [128, NCBLK, T] f16

        def emit_x_load(b):
            t = xf16_pool.tile([P, NTBLK, C], F16, tag="xf16", name="xf16t")
            nc.sync.dma_start(
                t[:], x[b].rearrange("(tb p) c -> p tb c", p=P)
            )
            xf16[b] = t

        def emit_x_transp(b, tbs):
            # PE fp16 transpose blocks (4 per tb into one psum bank), one
            # strided DVE evac per tb into the xt layout.
            if b not in xt:
                xt[b] = xt_pool.tile([P, NCBLK, T], F16, tag="xt", name="xtt")
            for tb in tbs:
                tps = mm_psum.tile([P, NCBLK, P], F16, tag="mm", name="tps")
                for cb in range(NCBLK):
                    nc.tensor.transpose(
                        tps[:, cb, :],
                        xf16[b][:, tb, cb * P : (cb + 1) * P],
                        identity16[:],
                    )
                nc.vector.tensor_copy(
                    xt[b][:, :, tb * P : (tb + 1) * P], tps[:]
                )

        def emit_qkv_group(b, k):
            # k in 0..7: QK psum group; 8..11: V group (tb = k-8)
            if b not in qt:
                qt[b] = qt_pool.tile([P, NHG, T], F16, tag="qt", name="qtt")
                kt[b] = kt_pool.tile([P, NHG, T], F16, tag="kt", name="ktt")
            if k < 8:
                ps = mm_psum.tile([P, T], F32, tag="mm", name="qkps")
                for co in range(NCBLK):
                    nc.tensor.matmul(
                        ps[:],
                        wqk[:, co, k * P : (k + 1) * P],
                        xt[b][:, co, :],
                        start=(co == 0),
                        stop=(co == NCBLK - 1),
                    )
                dst = qt[b][:, k, :] if k < NHG else kt[b][:, k - NHG, :]
                if k % 2 == 1:
                    nc.scalar.activation(
                        dst, ps[:], AF.Identity, bias=battn[:, k : k + 1]
                    )
                else:
                    nc.vector.tensor_scalar_add(dst, ps[:], battn[:, k : k + 1])
            else:
                tb = k - 8
                ps = mm_psum.tile([P, C], F32, tag="mm", name="vps")
                for co in range(NCBLK):
                    nc.tensor.matmul(
                        ps[:],
                        xt[b][:, co, tb * P : (tb + 1) * P],
                        wv[:, co, :],
                        start=(co == 0),
                        stop=(co == NCBLK - 1),
                    )
                v_buf = v_bufs[b % 2]
                psr = ps.rearrange("p (g i d) -> p g i d", g=NHG, i=4)
                bvr = bv_bcast.rearrange("p (g i d) -> p g i d", g=NHG, i=4)
                nc.vector.tensor_tensor(
                    v_buf[:, tb, :, 0:2, 0:D], psr[:, :, 0:2, :],
                    bvr[:, :, 0:2, :], ALU.add,
                )
                nc.vector.tensor_tensor(
                    v_buf[:, tb, :, 2:4, D : 2 * D], psr[:, :, 2:4, :],
                    bvr[:, :, 2:4, :], ALU.add,
                )

        def emit_scores_i(b, g, i):
            # 4-way row-packed K=32 scores matmuls for block i, then exp
            # (2 heads per op) and the diagonal causal mask (all 4 heads).
            q0 = QR0[i]
            n = T - q0
            s01 = s_psum.tile([P, 2, T], F32, tag="s", name="s01")
            s23 = s_psum.tile([P, 2, T], F32, tag="s", name="s23")
            for idx in range(4):
                st = s01 if idx < 2 else s23
                nc.tensor.matmul(
                    st[:, idx % 2, :n],
                    kt[b][32 * idx : 32 * idx + 32, g, i * P : (i + 1) * P],
                    qt[b][32 * idx : 32 * idx + 32, g, q0:T],
                    start=True,
                    stop=True,
                    tile_position=(32 * idx, 0),
                )
            ept = ep_pools[i].tile([P, 4, n], F16, tag=f"ep{i}", name="ept")
            nc.scalar.activation(
                ept[:, 0:2, :], s01[:, :, :n], AF.Exp, scale=SCALE, bias=ebias[:]
            )
            nc.scalar.activation(
                ept[:, 2:4, :], s23[:, :, :n], AF.Exp, scale=SCALE, bias=ebias[:]
            )
            # causal mask on the diagonal block (local cols [0, 128)):
            # split across DVE (tri multiply) and GpSimd (affine_select)
            # to balance the two engines.
            if i < 2:
                nc.vector.tensor_tensor(
                    ept[:, :, 0:P], ept[:, :, 0:P], tri4[:], ALU.mult
                )
            else:
                nc.gpsimd.affine_select(
                    out=ept[:, :, 0:P],
                    in_=ept[:, :, 0:P],
                    compare_op=ALU.is_ge,
                    fill=0.0,
                    base=0,
                    channel_multiplier=-1,
                    pattern=[[0, 4], [1, P]],
                )
            ep[(b, g, i)] = ept

        def emit_yt_pair(b, g, pair):
            # pair 0: heads (g,0)&(g,2); pair 1: heads (g,1)&(g,3).
            # lo=0 stationary [v|ones] -> rows 0-31 yT_A, 32-63 den_A;
            # lo=1 stationary [ones|v] -> rows 64-95 den_B, 96-127 yT_B.
            v_buf = v_bufs[b % 2]
            t = y_psum.tile([P, T], F32, tag="y", name="yps")
            for i in range(NTBLK):
                for lo, idx in ((0, pair), (1, pair + 2)):
                    nc.tensor.matmul(
                        t[64 * lo : 64 * lo + 64, QR0[i] : T],
                        v_buf[:, i, g, idx, :],
                        ep[(b, g, i)][:, idx, :],
                        start=(i == 0),
                        stop=(i == NTBLK - 1),
                        tile_position=(0, 64 * lo),
                        skip_group_check=True,
                    )
            yps[(b, g, pair)] = t

        def emit_evac_recip(b, g):
            # evac both pairs' yps to fp16 + paired reciprocal of the
            # adjacent denominator rows 63/64 (the custom DVE op must
            # keep in/out on the same partitions, hence the [66, T] rec
            # tile), then one broadcast DMA per pair replicates each rec
            # row to 32 partitions of the [128, T] bcast tile.
            for pair in (0, 1):
                t = yps.pop((b, g, pair))
                st = staged_pool.tile([P, T], F16, tag="staged", name="stt")
                nc.vector.tensor_copy(st[:], t[:])
                # full-tile reciprocal (the custom DVE op needs identical
                # base-0 in/out partition ranges; cost is free-size-based
                # so the extra rows are free).  Rows 63/64 hold 1/den.
                r = rec_pool.tile([P, T], F32, tag="rec", name="rt")
                nc.vector.reciprocal_approx_fast(r[:, :], t[:, :])
                staged[(b, g, pair)] = st
                # bcast rows aligned with the staged yT rows (DVE needs
                # equal base partitions when both inputs are SBUF):
                # rows 0-31 <- 1/den_A, rows 96-127 <- 1/den_B.
                bc = bcast_pool.tile([P, T], F32, tag="bcast", name="bct")
                for lo, row in ((0, 63), (96, 64)):
                    src = (
                        r[row : row + 1, :]
                        .rearrange("r (a t) -> r a t", a=1)
                        .to_broadcast([1, 32, T])
                    )
                    nc.sync.dma_start(bc[lo : lo + 32, :], src)
                bcast[(b, g, pair)] = bc

        def emit_mults(b, g):
            # normalize yT by 1/den and write straight into ytn layout.
            # rec16 rows: 0 -> head (g,0), 1 -> (g,2), 2 -> (g,1), 3 -> (g,3)
            if b not in ytn:
                ytn[b] = ytn_pool.tile([P, NCBLK, T], F16, tag="ytn", name="ytnt")
            yt = ytn[b]
            bc0 = bcast[(b, g, 0)]
            bc1 = bcast[(b, g, 1)]
            st0 = staged[(b, g, 0)]
            st1 = staged[(b, g, 1)]
            # all normalize multiplies on GpSimd (SBUF-only operands);
            # DVE is the saturated engine.
            nc.gpsimd.tensor_tensor(
                yt[0:32, g, :], st0[0:32, :], bc0[0:32, :], ALU.mult
            )
            nc.gpsimd.tensor_tensor(
                yt[64:96, g, :], st0[96:128, :], bc0[96:128, :], ALU.mult
            )
            nc.gpsimd.tensor_tensor(
                yt[32:64, g, :], st1[0:32, :], bc1[0:32, :], ALU.mult
            )
            nc.gpsimd.tensor_tensor(
                yt[96:128, g, :], st1[96:128, :], bc1[96:128, :], ALU.mult
            )
            del staged[(b, g, 0)], staged[(b, g, 1)]
            del bcast[(b, g, 0)], bcast[(b, g, 1)]

        def emit_proj(b, tb):
            ps = mm_psum.tile([P, C], F32, tag="mm", name="ops")
            for cb in range(NCBLK):
                nc.tensor.matmul(
                    ps[:],
                    ytn[b][:, cb, tb * P : (tb + 1) * P],
                    wp[:, cb, :],
                    start=(cb == 0),
                    stop=(cb == NCBLK - 1),
                )
            ob = o_pool.tile([P, C], F32, tag="obuf")
            nc.vector.tensor_tensor(ob[:], ps[:], bp_bcast[:], ALU.add)
            nc.sync.dma_start(out[b, tb * P : (tb + 1) * P, :], ob[:])

        # ---------------- prologue ----------------
        emit_x_load(0)
        emit_x_transp(0, [0, 1, 2, 3])
        emit_x_load(1)
        emit_x_transp(1, [0, 1, 2, 3])
        for k in range(NQKV):
            emit_qkv_group(0, k)

        # ---------------- main pipeline ----------------
        # stage s = 4*b + g; at step s:
        #   scores/exp/mask (s) | qkv(b+1) | yT (s-1) | evac+recip (s-1)
        #   mults (s-2) | proj(b-1) at g in {2,3} | x machinery for b+2
        n_steps = 4 * b_count

        def run_step(s):
            b, g = divmod(s, 4)
            pb, pg = divmod(s - 1, 4)   # previous stage (yT / evac)
            mb, mg = divmod(s - 2, 4)   # mults stage

            qkv_sched = {0: [0, 1, 2], 1: [3, 4, 5], 2: [6, 7, 8], 3: [9, 10, 11]}
            chunks = qkv_sched[g] if b + 1 < b_count else []

            # PE order: each scores pack is separated by >=1.5us of
            # independent PE work so the exp WAR (2 s-tile psum rotation)
            # never blocks the in-order PE queue.
            emit_scores_i(b, g, 0)
            if chunks:
                emit_qkv_group(b + 1, chunks[0])
                emit_qkv_group(b + 1, chunks[1])
            emit_scores_i(b, g, 1)
            if chunks:
                emit_qkv_group(b + 1, chunks[2])
            if b >= 2:
                emit_proj(b - 2, g)
            emit_scores_i(b, g, 2)
            if s >= 1:
                emit_yt_pair(pb, pg, 0)
            emit_scores_i(b, g, 3)
            if s >= 1:
                emit_yt_pair(pb, pg, 1)
                emit_evac_recip(pb, pg)
                if pg == 3:
                    for i in range(NTBLK):
                        del ep[(pb, 3, i)]
            if s >= 2:
                emit_mults(mb, mg)
            if g == 0 and b + 2 < b_count:
                emit_x_load(b + 2)
            if g == 2 and b + 2 < b_count:
                emit_x_transp(b + 2, [0, 1])
            if g == 3 and b + 2 < b_count:
                emit_x_transp(b + 2, [2, 3])
            if pg == 3 and pb >= 0:
                del qt[pb], kt[pb]

        for s in range(n_steps):
            run_step(s)

        # ---------------- epilogue ----------------
        lb = b_count - 1
        emit_yt_pair(lb, 3, 0)
        emit_yt_pair(lb, 3, 1)
        emit_evac_recip(lb, 3)
        emit_mults(lb, 2)
        emit_mults(lb, 3)
        for tb in range(NTBLK):
            emit_proj(lb - 1, tb)
        for tb in range(NTBLK):
            emit_proj(lb, tb)


_NC_CACHE = None


def build_nc(b_count: int = BC, num_devices: int = N_CORES):
    nc = bacc.Bacc(
        "TRN2", target_bir_lowering=False, debug=False, num_devices=num_devices
    )
    x = nc.dram_tensor("x", [b_count, T, C], F16, kind="ExternalInput").ap()
    w_attn = nc.dram_tensor("w_attn", [C, 3 * C], F32, kind="ExternalInput").ap()
    b_attn = nc.dram_tensor("b_attn", [3 * C], F32, kind="ExternalInput").ap()
    w_proj = nc.dram_tensor("w_proj", [C, C], F32, kind="ExternalInput").ap()
    b_proj = nc.dram_tensor("b_proj", [C], F32, kind="ExternalInput").ap()
    out = nc.dram_tensor("out", [b_count, T, C], F32, kind="ExternalOutput").ap()
    with tile.TileContext(nc) as tc:
        _build_kernel(tc, out, x, w_attn, b_attn, w_proj, b_proj, b_count)
    nc.compile()
    return nc


def _get_nc():
    global _NC_CACHE
    if _NC_CACHE is None:
        _NC_CACHE = build_nc(BC, N_CORES)
    return _NC_CACHE


def kernel(x, W_attn, b_attn, W_proj, b_proj):
    # host-side cast: the device consumes x in fp16 (matmul operand
    # precision) so there is no on-chip cast or fp32 x traffic at all.
    x = np.ascontiguousarray(np.asarray(x, dtype=np.float16))
    W_attn = np.ascontiguousarray(np.asarray(W_attn, dtype=np.float32))
    b_attn = np.ascontiguousarray(np.asarray(b_attn, dtype=np.float32))
    W_proj = np.ascontiguousarray(np.asarray(W_proj, dtype=np.float32))
    b_proj = np.ascontiguousarray(np.asarray(b_proj, dtype=np.float32))

    nc = _get_nc()
    in_maps = [
        {
            "x": x[c * BC : (c + 1) * BC],
            "w_attn": W_attn,
            "b_attn": b_attn,
            "w_proj": W_proj,
            "b_proj": b_proj,
        }
        for c in range(N_CORES)
    ]
    res = run_bass_kernel_spmd(nc, in_maps, core_ids=list(range(N_CORES)))
    return np.concatenate([res.results[c]["out"] for c in range(N_CORES)], axis=0)
